# revision 1
# baseline (speedup 1.0000x reference)
"""Trainium2 Bass kernel for the channel-attention module.

Reference computation (per batch item, C=256 channels, N=4096 pixels):
    q = wq@x + bq; k = wk@x + bk; v = wv@x + bv          (1x1 convs)
    energy = q @ k^T                 [C, C]
    attn = softmax(energy, -1)
    out = attn @ v                   [C, N]
    y = gamma*out + x

Algorithm used here (algebraically identical, far less PE work):
    G' = [[x x^T, s], [s^T, N]]  (s = row sums of x)  -- Gram matrix, 257x257
    energy = wq' G' wk'^T   where wq' = [wq | bq], wk' = [wk | bk]
    attn = softmax(energy)
    out = (attn wv) x + (attn bv) 1^T
    y = gamma*out + x

This eliminates materializing q, k, v entirely: the only O(C*C*N) work is
the Gram matrix, the transpose of x it needs, and the final (attn wv) x.

Sharding: data-parallel over batch B=16 across 8 cores (2 items/core).

Matmul dtype: float16 (10-bit mantissa like TF32, fp32 PSUM accumulation,
full PE rate + fast weight load). Measured end-to-end error vs the fp32
reference: ~3e-4 (fro). All intermediates stay well inside fp16 range
(|G|<4.4e3, |energy|<450).
"""

import os
import sys

sys.path.insert(0, "/opt/trn_rl_repo")

from contextlib import ExitStack

import numpy as np

import concourse.bacc as bacc
import concourse.tile as tile
from concourse import masks, mybir
from concourse.bass_utils import run_bass_kernel_spmd

F32 = mybir.dt.float32
F16 = mybir.dt.float16

B, C, H, W = 16, 256, 64, 64
N = H * W                 # 4096
NCORES = 8
PB = B // NCORES          # batch items per core
P = 128                   # partitions
CT = C // P               # 2 channel tiles
NT = N // P               # 32 pixel tiles
FD = 512                  # free-dim chunk for the final matmul
YCOLS = 2048              # output staging width (1 MiB per DMA)

# wpack column layout (fp16, packed on host into [128, WCOLS]):
_WQ0, _WQ1 = 0, 256              # wq^T rows 0:128 / 128:256   [128,256] each
_WK0, _WK1 = 512, 768            # wk^T rows 0:128 / 128:256
_WV0, _WV1 = 1024, 1282         # [wv | bv | 0] rows 0:128 / 128:256 [128,258]
_BQ = 1540                       # rows 0:2: [bq; 0]            [2,256]
_BK = 1796                       # rows 0:2: [bk; 0]            [2,256]
_GA = 2052                       # gamma replicated             [128,1]
_ONE = 2053                      # col of 1.0 (all rows), col of 0.0
_NC = 2055                       # row 0: float(N) = 4096.0
WCOLS = 2056


def _emit_core_program(nc, tc, ctx, x_in, wpack, y_out):
    sb1 = ctx.enter_context(tc.tile_pool(name="sb1", bufs=1))
    xbp = ctx.enter_context(tc.tile_pool(name="xbp", bufs=2 * PB))
    xtp = ctx.enter_context(tc.tile_pool(name="xtp", bufs=2))
    gsb = ctx.enter_context(tc.tile_pool(name="gsb", bufs=4))
    smp = ctx.enter_context(tc.tile_pool(name="smp", bufs=8))
    ysp = ctx.enter_context(tc.tile_pool(name="ysp", bufs=4))
    # PSUM: 2+2+2+2 = 8 banks
    pst = ctx.enter_context(tc.tile_pool(name="pst", bufs=2, space="PSUM"))
    psg = ctx.enter_context(tc.tile_pool(name="psg", bufs=2, space="PSUM"))
    pss = ctx.enter_context(tc.tile_pool(name="pss", bufs=2, space="PSUM"))
    pso = ctx.enter_context(tc.tile_pool(name="pso", bufs=2, space="PSUM"))

    # --- constants: packed weights (one DMA) + identity ---
    wt = sb1.tile([P, WCOLS], F16)
    nc.sync.dma_start(out=wt, in_=wpack[:, :])
    ident_f = sb1.tile([P, P], F32)
    masks.make_identity(nc, ident_f[:, :])
    ident = sb1.tile([P, P], F16)
    nc.vector.tensor_copy(ident, ident_f)

    gamma_col = sb1.tile([P, 1], F32, name="gamma_col")
    nc.vector.tensor_copy(gamma_col, wt[:, _GA:_GA + 1])

    wq_k = [wt[:, _WQ0:_WQ0 + 256], wt[:, _WQ1:_WQ1 + 256],
            wt[0:2, _BQ:_BQ + 256]]
    wk_k = [wt[:, _WK0:_WK0 + 256], wt[:, _WK1:_WK1 + 256],
            wt[0:2, _BK:_BK + 256]]
    wv_t = [wt[:, _WV0:_WV0 + 258], wt[:, _WV1:_WV1 + 258]]

    st = [dict() for _ in range(PB)]

    # ---- phase A: loads + x^T DMA-transpose + Gram + s (both items) ----
    for b in range(PB):
        s = st[b]
        xb = []
        for ct in range(CT):
            t = xbp.tile([P, N], F16, tag="xb", name=f"xb{b}_{ct}")
            nc.sync.dma_start(out=t, in_=x_in[b, ct * P:(ct + 1) * P, :])
            xb.append(t)
        s["xb"] = xb
        with nc.named_scope("gram"):
            xt = xtp.tile([P, NT * 256], F16, tag="xt", name=f"xt{b}")
            xt3 = xt.rearrange("p (t c) -> p t c", c=256)
            for gch in range(4):
                eng = nc.sync if gch % 2 == 0 else nc.scalar
                eng.dma_start_transpose(
                    xt3[:, gch * (NT // 4):(gch + 1) * (NT // 4), :],
                    x_in[b, :, gch * (N // 4):(gch + 1) * (N // 4)])
            s["xt"] = xt
            # s column: DVE reduce for ct0, ACT accum trick for ct1
            scol = []
            for ct in range(CT):
                scf = smp.tile([P, 1], F32, tag="scf", name=f"scf{b}_{ct}")
                nc.vector.tensor_reduce(
                    scf, xb[ct], axis=mybir.AxisListType.X,
                    op=mybir.AluOpType.add)
                sc = smp.tile([P, 2], F16, tag="scol", name=f"scol{b}_{ct}")
                nc.vector.tensor_copy(sc[:, 0:1], scf)
                nc.vector.tensor_copy(sc[:, 1:2], wt[:, _ONE + 1:_ONE + 2])
                scol.append(sc)
            s["scol"] = scol
            gps = [psg.tile([P, 256], F32, tag="gacc", name=f"gps{b}_{i}")
                   for i in range(CT)]
            for nt in range(NT):
                for ct in range(CT):
                    nc.tensor.matmul(
                        gps[ct],
                        xt[:, nt * 256 + ct * P:nt * 256 + (ct + 1) * P],
                        xt[:, nt * 256:(nt + 1) * 256],
                        start=(nt == 0), stop=(nt == NT - 1))
            g = []
            for ct in range(CT):
                gt = gsb.tile([P, 256], F16, tag="g", name=f"g{b}_{ct}")
                nc.vector.tensor_copy(gt, gps[ct])
                g.append(gt)
            # srow [2,256] = [s^T; 0] via PE transpose of scol tiles
            srow = gsb.tile([2, 256], F16, tag="srow", name=f"srow{b}")
            for ct in range(CT):
                sp = pst.tile([2, P], F16, tag="tp", name=f"sp{b}_{ct}")
                nc.tensor.transpose(sp, scol[ct], ident)
                nc.vector.tensor_copy(srow[0:2, ct * P:(ct + 1) * P], sp)
            g2 = gsb.tile([2, 257], F16, tag="g2", name=f"g2{b}")
            nc.vector.tensor_copy(g2[0:2, 0:256], srow[0:2, :])
            nc.vector.tensor_copy(g2[0:2, 256:257], wt[0:2, _NC:_NC + 1])
            s["gk"] = [g[0], g[1], g2]

    # ---- phase B1: T^T and E (both items) ----
    for b in range(PB):
        s = st[b]
        gk, scol = s["gk"], s["scol"]
        with nc.named_scope("energy"):
            ttp = [pss.tile([P, 256], F32, tag="big", name=f"ttp{b}_{i}")
                   for i in range(CT)]
            ttp.append(pss.tile([2, 256], F32, tag="big", name=f"ttp{b}_2"))
            for mt in range(CT):          # output rows m 0:128 / 128:256
                for kt in range(3):       # contraction over p
                    lhs = gk[kt][:, mt * P:(mt + 1) * P]
                    nc.tensor.matmul(ttp[mt], lhs, wq_k[kt],
                                     start=(kt == 0), stop=(kt == 2))
            for kt in range(3):           # output rows 256:257 (lhsT = s col)
                lhs = scol[kt][:, 0:1] if kt < 2 else gk[2][:, 256:257]
                nc.tensor.matmul(ttp[2][0:1, :], lhs, wq_k[kt],
                                 start=(kt == 0), stop=(kt == 2))
            tt = []
            for mt in range(CT):
                t = gsb.tile([P, 256], F16, tag="tt", name=f"tt{b}_{mt}")
                nc.vector.tensor_copy(t, ttp[mt])
                tt.append(t)
            t2 = gsb.tile([1, 256], F16, tag="tt2", name=f"tt{b}_2")
            nc.vector.tensor_copy(t2, ttp[2][0:1, :])
            tt.append(t2)

            ep = pss.tile([P, 2 * 256], F32, tag="big", name=f"ep{b}")
            for it in range(CT):
                for kt in range(3):
                    lhs = tt[kt][:, it * P:(it + 1) * P] if kt < 2 \
                        else tt[2][0:1, it * P:(it + 1) * P]
                    nc.tensor.matmul(ep[:, it * 256:(it + 1) * 256],
                                     lhs, wk_k[kt][0:1, :] if kt == 2
                                     else wk_k[kt],
                                     start=(kt == 0), stop=(kt == 2))
            s["ep"] = ep

    # ---- phase B2: softmax + attn^T + A (both items) ----
    for b in range(PB):
        s = st[b]
        ep = s["ep"]
        with nc.named_scope("softmax"):
            attn = []
            for it in range(CT):
                eslice = ep[:, it * 256:(it + 1) * 256]
                nmx = smp.tile([P, 1], F32, tag="nmx", name=f"nmx{b}_{it}")
                nc.vector.tensor_reduce(
                    nmx, eslice, axis=mybir.AxisListType.X,
                    op=mybir.AluOpType.max, negate=True)
                at = smp.tile([P, 256], F16, tag="attn", name=f"at{b}_{it}")
                rs = smp.tile([P, 1], F32, tag="rs", name=f"rs{b}_{it}")
                nc.scalar.activation(
                    out=at, in_=eslice,
                    func=mybir.ActivationFunctionType.Exp,
                    bias=nmx, scale=1.0, accum_out=rs)
                ri = smp.tile([P, 1], F32, tag="ri", name=f"ri{b}_{it}")
                nc.vector.reciprocal(ri, rs)
                nc.vector.tensor_scalar_mul(at, at, ri)
                attn.append(at)
            attnT = []
            for jt in range(CT):
                aT = smp.tile([P, 256], F16, tag="attnT", name=f"aT{b}_{jt}")
                for it in range(CT):
                    tp = pst.tile([P, P], F16, tag="tp", name=f"tA{b}{jt}{it}")
                    nc.tensor.transpose(
                        tp, attn[it][:, jt * P:(jt + 1) * P], ident)
                    nc.vector.tensor_copy(aT[:, it * P:(it + 1) * P], tp)
                attnT.append(aT)
        with nc.named_scope("attn_wv"):
            ap_ = pss.tile([P, 2 * 256], F32, tag="big", name=f"ap{b}")
            for mt in range(CT):
                for jt in range(CT):
                    nc.tensor.matmul(
                        ap_[:, mt * 256:(mt + 1) * 256],
                        wv_t[jt][:, mt * P:(mt + 1) * P], attnT[jt],
                        start=(jt == 0), stop=(jt == 1))
            arow = pss.tile([2, 256], F32, tag="big", name=f"arow{b}")
            for jt in range(CT):
                nc.tensor.matmul(arow, wv_t[jt][:, 256:258], attnT[jt],
                                 start=(jt == 0), stop=(jt == 1))
            at_s = []
            for mt in range(CT):
                t = gsb.tile([P, 256], F16, tag="ats", name=f"ats{b}_{mt}")
                nc.vector.tensor_scalar_mul(
                    t, ap_[:, mt * 256:(mt + 1) * 256], gamma_col)
                nc.vector.tensor_add(
                    t[:, mt * P:(mt + 1) * P], t[:, mt * P:(mt + 1) * P],
                    ident)
                at_s.append(t)
            s["at_s"] = at_s
            abv_r = smp.tile([2, 256], F16, tag="abvr", name=f"abvr{b}")
            nc.vector.tensor_scalar_mul(abv_r, arow, gamma_col[0:2, :])
            gabv = []
            for it in range(CT):
                tp = pst.tile([P, 2], F16, tag="tp", name=f"tb{b}_{it}")
                nc.tensor.transpose(
                    tp, abv_r[0:2, it * P:(it + 1) * P], ident[0:2, 0:2])
                gc = smp.tile([P, 1], F32, tag="gabv", name=f"gabv{b}_{it}")
                nc.vector.tensor_copy(gc, tp[:, 0:1])
                gabv.append(gc)
            s["gabv"] = gabv

    # ---- phase C: out = B x (+ gamma*abv bias), store (both items) ----
    for b in range(PB):
        s = st[b]
        xb, at_s, gabv = s["xb"], s["at_s"], s["gabv"]
        with nc.named_scope("out_mm"):
            for it in range(CT):
                for yg in range(N // YCOLS):
                    ysb = ysp.tile([P, YCOLS], F32, tag="ysb",
                                   name=f"ysb{b}_{it}_{yg}")
                    for sub in range(YCOLS // FD):
                        nch = yg * (YCOLS // FD) + sub
                        op = pso.tile([P, FD], F32, tag="out",
                                      name=f"op{b}_{it}_{nch}")
                        for ct in range(CT):
                            nc.tensor.matmul(
                                op, at_s[ct][:, it * P:(it + 1) * P],
                                xb[ct][:, nch * FD:(nch + 1) * FD],
                                start=(ct == 0), stop=(ct == CT - 1))
                        nc.vector.tensor_scalar_add(
                            ysb[:, sub * FD:(sub + 1) * FD], op, gabv[it])
                    nc.sync.dma_start(
                        out=y_out[b, it * P:(it + 1) * P,
                                  yg * YCOLS:(yg + 1) * YCOLS],
                        in_=ysb)


_CACHE = {}
LAST_RESULTS = None


def _build():
    if "nc" in _CACHE:
        return _CACHE["nc"]
    nc = bacc.Bacc()
    x_in = nc.declare_dram_parameter("x", [PB, C, N], F16, isOutput=False)
    wpack = nc.declare_dram_parameter("wpack", [P, WCOLS], F16,
                                      isOutput=False)
    y_out = nc.declare_dram_parameter("y", [PB, C, N], F32, isOutput=True)
    with ExitStack() as ctx:
        tc = ctx.enter_context(tile.TileContext(nc))
        _emit_core_program(nc, tc, ctx, x_in, wpack, y_out)
    nc.compile()
    _CACHE["nc"] = nc
    return nc


def _pack_weights(wq, bq, wk, bk, wv, bv, gamma):
    wp = np.zeros((P, WCOLS), np.float16)
    wqT = np.ascontiguousarray(wq.T).astype(np.float16)
    wkT = np.ascontiguousarray(wk.T).astype(np.float16)
    wp[:, _WQ0:_WQ0 + 256] = wqT[0:P]
    wp[:, _WQ1:_WQ1 + 256] = wqT[P:C]
    wp[:, _WK0:_WK0 + 256] = wkT[0:P]
    wp[:, _WK1:_WK1 + 256] = wkT[P:C]
    wvp = np.concatenate([wv, bv[:, None]],
                         axis=1).astype(np.float16)  # [256, 257]
    wp[:, _WV0:_WV0 + 257] = wvp[0:P]
    wp[:, _WV1:_WV1 + 257] = wvp[P:C]
    wp[0, _BQ:_BQ + 256] = bq.astype(np.float16)
    wp[0, _BK:_BK + 256] = bk.astype(np.float16)
    wp[:, _GA] = np.float16(gamma)
    wp[:, _ONE] = np.float16(1.0)
    wp[0, _NC] = np.float16(float(N))
    return wp


def kernel(x, wq, bq, wk, bk, wv, bv, gamma):
    global LAST_RESULTS
    x = np.asarray(x, np.float32)
    x16 = np.ascontiguousarray(x.reshape(B, C, N).astype(np.float16))
    wp = _pack_weights(np.asarray(wq, np.float32), np.asarray(bq, np.float32),
                       np.asarray(wk, np.float32), np.asarray(bk, np.float32),
                       np.asarray(wv, np.float32), np.asarray(bv, np.float32),
                       np.asarray(gamma, np.float32).reshape(-1)[0])
    nc = _build()
    in_maps = []
    for k in range(NCORES):
        in_maps.append({
            "x": np.ascontiguousarray(x16[k * PB:(k + 1) * PB]),
            "wpack": wp,
        })
    trace = bool(int(os.environ.get("KERNEL_TRACE", "0")))
    res = run_bass_kernel_spmd(nc, in_maps, core_ids=list(range(NCORES)),
                               trace=trace)
    LAST_RESULTS = res
    y = np.concatenate([res.results[k]["y"][None] for k in range(NCORES)],
                       axis=0)
    return y.reshape(B, C, H, W).astype(np.float32)



# revision 6
# speedup vs baseline: 1.3868x; 1.3868x over previous
"""Trainium2 Bass kernel for the channel-attention module.

Reference computation (per batch item, C=256 channels, N=4096 pixels):
    q = wq@x + bq; k = wk@x + bk; v = wv@x + bv          (1x1 convs)
    energy = q @ k^T                 [C, C]
    attn = softmax(energy, -1)
    out = attn @ v                   [C, N]
    y = gamma*out + x

Algorithm (algebraically identical, far less PE work):
    G' = [[x x^T, s], [s^T, N]]  (s = row sums of x)  -- Gram matrix, 257x257
    energy = wq' G' wk'^T   where wq' = [wq | bq], wk' = [wk | bk]
    attn = softmax(energy)
    out_dev = (attn wv) x + (attn bv) 1^T      (returned unscaled, fp16)
    y = gamma*out_dev + x                       (host, fp32)

Key layout/precision choices:
  * Host supplies x^T pre-transposed with a ones column appended, so the
    Gram matmul also produces the row sums s (no DVE reduce, no DMA
    transpose on device).
  * Gram is symmetric: row-block 1 is computed only for cols 128:258 and
    the (1,0) block is reconstructed with one PE transpose.
  * The final (attn wv) @ x matmul runs in fp8-e4m3 with DoubleRow perf
    mode (256-deep contraction per instruction, 2x PE rate). x is sent
    as a separate fp8 copy. The x passthrough and gamma scaling happen
    on the host in fp32, so fp8 never touches the dominant x term.
  * energy path stays fp16 (measured end-to-end error ~3e-4).

Sharding: data-parallel over batch B=16 across 8 cores (2 items/core).
"""

import os
import sys

sys.path.insert(0, "/opt/trn_rl_repo")

from contextlib import ExitStack

import ml_dtypes
import numpy as np

import concourse.bacc as bacc
import concourse.tile as tile
from concourse import masks, mybir
from concourse.bass_utils import run_bass_kernel_spmd

F32 = mybir.dt.float32
F16 = mybir.dt.float16
F8 = mybir.dt.float8e4

B, C, H, W = 16, 256, 64, 64
N = H * W                 # 4096
NCORES = 8
PB = B // NCORES          # batch items per core
P = 128                   # partitions
CT = C // P               # 2 channel tiles
NT = N // P               # 32 pixel tiles
CC = 258                  # per-pixel-tile row width: 256 ch + [1, 0]
NCH = 4                   # xt DMA chunks per item
NTC = NT // NCH           # pixel tiles per chunk (8)
FD = 512                  # free-dim chunk for the final matmul
YCOLS = 2048              # output staging width (512 KiB per DMA)

# wpack column layout (fp16, packed on host into [128, WCOLS]):
_WQ0, _WQ1 = 0, 256              # wq^T rows 0:128 / 128:256   [128,256] each
_WK0, _WK1 = 512, 768            # wk^T rows 0:128 / 128:256
_WV0, _WV1 = 1024, 1282          # [wv | bv | 0] rows 0:128 / 128:256 [128,258]
_BQ = 1540                       # rows 0:2: [bq; 0]            [2,256]
_BK = 1796                       # rows 0:2: [bk; 0]            [2,256]
_NC = 2052                       # rows 0:2: [float(N); 0]      [2,1]
WCOLS = 2056


def _emit_core_program(nc, tc, ctx, xt_in, x8_in, wpack, y_out):
    sb1 = ctx.enter_context(tc.tile_pool(name="sb1", bufs=1))
    xtp = ctx.enter_context(tc.tile_pool(name="xtp", bufs=NCH * PB))
    x8p = ctx.enter_context(tc.tile_pool(name="x8p", bufs=PB))
    gsb = ctx.enter_context(tc.tile_pool(name="gsb", bufs=2 * PB))
    smp = ctx.enter_context(tc.tile_pool(name="smp", bufs=10))
    ysp = ctx.enter_context(tc.tile_pool(name="ysp", bufs=4))
    # PSUM pools: 1 + 2 + 2 + 3 = 8 banks
    psg = ctx.enter_context(tc.tile_pool(name="psg", bufs=1, space="PSUM"))
    psb = ctx.enter_context(tc.tile_pool(name="psb", bufs=2, space="PSUM"))
    pss = ctx.enter_context(tc.tile_pool(name="pss", bufs=2, space="PSUM"))
    pso = ctx.enter_context(tc.tile_pool(name="pso", bufs=3, space="PSUM"))

    # --- constants: packed weights (one DMA) + identity ---
    wt = sb1.tile([P, WCOLS], F16)
    nc.sync.dma_start(out=wt, in_=wpack[:, :])
    ident_f = sb1.tile([P, P], F32)
    masks.make_identity(nc, ident_f[:, :])
    ident = sb1.tile([P, P], F16)
    nc.vector.tensor_copy(ident, ident_f)

    wq_k = [wt[:, _WQ0:_WQ0 + 256], wt[:, _WQ1:_WQ1 + 256],
            wt[0:2, _BQ:_BQ + 256]]
    wk_k = [wt[:, _WK0:_WK0 + 256], wt[:, _WK1:_WK1 + 256],
            wt[0:2, _BK:_BK + 256]]
    wv_t = [wt[:, _WV0:_WV0 + 258], wt[:, _WV1:_WV1 + 258]]

    st = [dict() for _ in range(PB)]

    # ---- input DMAs, all issued up front on separate queues ----
    for b in range(PB):
        s = st[b]
        eng = nc.sync if b == 0 else nc.gpsimd
        xt = []
        for ch in range(NCH):
            t = xtp.tile([P, NTC * CC], F16, tag="xt", name=f"xt{b}_{ch}")
            eng.dma_start(out=t,
                          in_=xt_in[b, :, ch * NTC * CC:(ch + 1) * NTC * CC])
            xt.append(t)
        s["xt"] = xt
        x8 = x8p.tile([P, CT, N], F8, tag="x8", name=f"x8_{b}")
        for ct in range(CT):
            nc.scalar.dma_start(out=x8[:, ct, :],
                                in_=x8_in[b, ct * P:(ct + 1) * P, :])
        s["x8"] = x8

    # ---- gram matmuls (PE), chunk-paced; row-block1 only cols 128:258 ----
    for b in range(PB):
        s = st[b]
        with nc.named_scope("gram"):
            # both accumulators packed into one PSUM bank (258 + 130 f32)
            gpsA = psg.tile([P, 402], F32, tag="g", name=f"gps{b}")
            gps0 = gpsA[:, 0:CC]
            gps1 = gpsA[:, 272:272 + CC - P]
            for nt in range(NT):
                ch, off = nt // NTC, nt % NTC
                xt = s["xt"][ch]
                lhs0 = xt[:, off * CC:off * CC + P]
                lhs1 = xt[:, off * CC + P:off * CC + 2 * P]
                rhs = xt[:, off * CC:(off + 1) * CC]
                rhs1 = xt[:, off * CC + P:(off + 1) * CC]
                nc.tensor.matmul(gps0, lhs0, rhs,
                                 start=(nt == 0), stop=(nt == NT - 1))
                nc.tensor.matmul(gps1, lhs1, rhs1,
                                 start=(nt == 0), stop=(nt == NT - 1))
            s["gps"] = (gps0, gps1)

    # ---- per-item tail: G assembly, T, E, softmax, attn_wv, out ----
    for b in range(PB):
        s = st[b]
        gps0, gps1 = s["gps"]
        with nc.named_scope("gass"):
            # g0 [128, 258] = G rows 0:128 (+ s col at 256, 0 at 257)
            g0 = gsb.tile([P, CC], F16, tag="g", name=f"g{b}_0")
            nc.vector.tensor_copy(g0, gps0)
            g1 = gsb.tile([P, CC], F16, tag="g", name=f"g{b}_1")
            nc.vector.tensor_copy(g1[:, P:CC], gps1)
            # block (1,0) = block (0,1)^T via PE transpose
            tp10 = pss.tile([P, P], F16, tag="sm", name=f"tp10_{b}")
            nc.tensor.transpose(tp10, g0[:, P:2 * P], ident)
            nc.vector.tensor_copy(g1[:, 0:P], tp10)
            # g2 [2, 257] = [[s^T, N], [0, 0]] via PE transpose of s cols
            g2 = gsb.tile([2, 257], F16, tag="g2", name=f"g2_{b}")
            for ct in range(CT):
                g = (g0, g1)[ct]
                sp = pss.tile([2, P], F16, tag="sm", name=f"sp{b}_{ct}")
                nc.tensor.transpose(sp, g[:, 256:258], ident)
                nc.vector.tensor_copy(g2[0:2, ct * P:(ct + 1) * P], sp)
            nc.vector.tensor_copy(g2[0:2, 256:257], wt[0:2, _NC:_NC + 1])
            gk = (g0, g1, g2)

        with nc.named_scope("energy"):
            # T^T = (wq' G')^T packed [128, 512]; row 256 separately
            ttp = psb.tile([P, 2 * 256], F32, tag="big", name=f"ttp{b}")
            for mt in range(CT):
                for kt in range(3):
                    lhs = gk[kt][:, mt * P:(mt + 1) * P] if kt < 2 \
                        else gk[2][0:2, mt * P:(mt + 1) * P]
                    nc.tensor.matmul(ttp[:, mt * 256:(mt + 1) * 256],
                                     lhs, wq_k[kt],
                                     start=(kt == 0), stop=(kt == 2))
            tt2p = pss.tile([1, 256], F32, tag="sm", name=f"tt2p{b}")
            for kt in range(3):
                lhs = gk[kt][:, 256:257] if kt < 2 else gk[2][0:2, 256:257]
                nc.tensor.matmul(tt2p, lhs, wq_k[kt],
                                 start=(kt == 0), stop=(kt == 2))
            tt = []
            for mt in range(CT):
                t = gsb.tile([P, 256], F16, tag="tt", name=f"tt{b}_{mt}")
                nc.vector.tensor_copy(t, ttp[:, mt * 256:(mt + 1) * 256])
                tt.append(t)
            t2 = gsb.tile([1, 256], F16, tag="tt2", name=f"tt2_{b}")
            nc.vector.tensor_copy(t2, tt2p)
            tt.append(t2)

            ep = psb.tile([P, 2 * 256], F32, tag="big", name=f"ep{b}")
            for it in range(CT):
                for kt in range(3):
                    lhs = tt[kt][:, it * P:(it + 1) * P] if kt < 2 \
                        else tt[2][0:1, it * P:(it + 1) * P]
                    nc.tensor.matmul(ep[:, it * 256:(it + 1) * 256],
                                     lhs, wk_k[kt][0:1, :] if kt == 2
                                     else wk_k[kt],
                                     start=(kt == 0), stop=(kt == 2))

        with nc.named_scope("softmax"):
            attn = []
            for it in range(CT):
                eslice = ep[:, it * 256:(it + 1) * 256]
                nmx = smp.tile([P, 1], F32, tag="nmx", name=f"nmx{b}_{it}")
                nc.vector.tensor_reduce(
                    nmx, eslice, axis=mybir.AxisListType.X,
                    op=mybir.AluOpType.max, negate=True)
                at = smp.tile([P, 256], F16, tag="attn", name=f"at{b}_{it}")
                rs = smp.tile([P, 1], F32, tag="rs", name=f"rs{b}_{it}")
                nc.scalar.activation(
                    out=at, in_=eslice,
                    func=mybir.ActivationFunctionType.Exp,
                    bias=nmx, scale=1.0, accum_out=rs)
                ri = smp.tile([P, 1], F32, tag="ri", name=f"ri{b}_{it}")
                nc.vector.reciprocal(ri, rs)
                nc.vector.tensor_scalar_mul(at, at, ri)
                attn.append(at)
            attnT = []
            for jt in range(CT):
                aT = smp.tile([P, 256], F16, tag="attnT", name=f"aT{b}_{jt}")
                attnT.append(aT)
            for it in range(CT):
                for jt in range(CT):
                    tp = pss.tile([P, P], F16, tag="sm", name=f"tA{b}{jt}{it}")
                    nc.tensor.transpose(
                        tp, attn[it][:, jt * P:(jt + 1) * P], ident)
                    nc.vector.tensor_copy(
                        attnT[jt][:, it * P:(it + 1) * P], tp)

        with nc.named_scope("attn_wv"):
            # at8[p, ct, o] = (attn wv)[o, ct*128+p] in fp8
            at8 = smp.tile([P, CT, 256], F8, tag="at8", name=f"at8_{b}")
            ap_ = psb.tile([P, 2 * 256], F32, tag="big", name=f"ap{b}")
            for mt in range(CT):
                for jt in range(CT):
                    nc.tensor.matmul(
                        ap_[:, mt * 256:(mt + 1) * 256],
                        wv_t[jt][:, mt * P:(mt + 1) * P], attnT[jt],
                        start=(jt == 0), stop=(jt == 1))
            for mt in range(CT):
                nc.vector.tensor_copy(at8[:, mt, :],
                                      ap_[:, mt * 256:(mt + 1) * 256])
            # abv[it] = (attn bv) column [128, 1]
            abv = []
            for it in range(CT):
                avp = pss.tile([P, 1], F32, tag="sm", name=f"avp{b}_{it}")
                for jt in range(CT):
                    nc.tensor.matmul(avp,
                                     attnT[jt][:, it * P:(it + 1) * P],
                                     wv_t[jt][:, 256:257],
                                     start=(jt == 0), stop=(jt == 1))
                ac = smp.tile([P, 1], F32, tag="abv", name=f"abv{b}_{it}")
                nc.vector.tensor_copy(ac, avp)
                abv.append(ac)

        # ---- out = (attn wv) x + abv (fp8 DoubleRow), store fp16 ----
        x8 = s["x8"]
        cpe = [nc.scalar, nc.vector, nc.scalar, nc.vector]
        with nc.named_scope("out_mm"):
            for it in range(CT):
                lhsT = at8[:, :, it * P:(it + 1) * P]
                for yg in range(N // YCOLS):
                    ysb = ysp.tile([P, YCOLS], F16, tag="ysb",
                                   name=f"ysb{b}_{it}_{yg}")
                    for sub in range(YCOLS // FD):
                        nch = yg * (YCOLS // FD) + sub
                        op = pso.tile([P, FD], F32, tag="out",
                                      name=f"op{b}_{it}_{nch}")
                        nc.tensor.matmul(
                            op, lhsT,
                            x8[:, :, nch * FD:(nch + 1) * FD],
                            start=True, stop=True,
                            perf_mode=mybir.MatmulPerfMode.DoubleRow)
                        eng = cpe[sub]
                        if eng is nc.scalar:
                            eng.add(ysb[:, sub * FD:(sub + 1) * FD], op,
                                    add=abv[it])
                        else:
                            eng.tensor_scalar_add(
                                ysb[:, sub * FD:(sub + 1) * FD], op, abv[it])
                    nc.sync.dma_start(
                        out=y_out[b, it * P:(it + 1) * P,
                                  yg * YCOLS:(yg + 1) * YCOLS],
                        in_=ysb)


_CACHE = {}
LAST_RESULTS = None


def _build():
    if "nc" in _CACHE:
        return _CACHE["nc"]
    nc = bacc.Bacc()
    xt_in = nc.declare_dram_parameter("xt", [PB, P, NT * CC], F16,
                                      isOutput=False)
    x8_in = nc.declare_dram_parameter("x8", [PB, C, N], F8, isOutput=False)
    wpack = nc.declare_dram_parameter("wpack", [P, WCOLS], F16,
                                      isOutput=False)
    y_out = nc.declare_dram_parameter("y", [PB, C, N], F16, isOutput=True)
    with ExitStack() as ctx:
        tc = ctx.enter_context(tile.TileContext(nc))
        _emit_core_program(nc, tc, ctx, xt_in, x8_in, wpack, y_out)
    nc.compile()
    _CACHE["nc"] = nc
    return nc


def _pack_weights(wq, bq, wk, bk, wv, bv):
    wp = np.zeros((P, WCOLS), np.float16)
    wqT = np.ascontiguousarray(wq.T).astype(np.float16)
    wkT = np.ascontiguousarray(wk.T).astype(np.float16)
    wp[:, _WQ0:_WQ0 + 256] = wqT[0:P]
    wp[:, _WQ1:_WQ1 + 256] = wqT[P:C]
    wp[:, _WK0:_WK0 + 256] = wkT[0:P]
    wp[:, _WK1:_WK1 + 256] = wkT[P:C]
    wvp = np.concatenate([wv, bv[:, None]],
                         axis=1).astype(np.float16)  # [256, 257]
    wp[:, _WV0:_WV0 + 257] = wvp[0:P]
    wp[:, _WV1:_WV1 + 257] = wvp[P:C]
    wp[0, _BQ:_BQ + 256] = bq.astype(np.float16)
    wp[0, _BK:_BK + 256] = bk.astype(np.float16)
    wp[0, _NC] = np.float16(float(N))
    return wp


def kernel(x, wq, bq, wk, bk, wv, bv, gamma):
    global LAST_RESULTS
    x = np.ascontiguousarray(np.asarray(x, np.float32).reshape(B, C, N))
    x16 = x.astype(np.float16)
    # xt[b, p, nt, c] = x[b, c, nt*128+p]; col 256 = 1.0, col 257 = 0.0
    xt = np.zeros((B, P, NT, CC), np.float16)
    xt[:, :, :, :256] = x16.reshape(B, C, NT, P).transpose(0, 3, 2, 1)
    xt[:, :, :, 256] = np.float16(1.0)
    xt = np.ascontiguousarray(xt.reshape(B, P, NT * CC))
    x8 = x.astype(ml_dtypes.float8_e4m3)
    wp = _pack_weights(np.asarray(wq, np.float32), np.asarray(bq, np.float32),
                       np.asarray(wk, np.float32), np.asarray(bk, np.float32),
                       np.asarray(wv, np.float32), np.asarray(bv, np.float32))
    nc = _build()
    in_maps = []
    for k in range(NCORES):
        in_maps.append({
            "xt": np.ascontiguousarray(xt[k * PB:(k + 1) * PB]),
            "x8": np.ascontiguousarray(x8[k * PB:(k + 1) * PB]),
            "wpack": wp,
        })
    trace = bool(int(os.environ.get("KERNEL_TRACE", "0")))
    res = run_bass_kernel_spmd(nc, in_maps, core_ids=list(range(NCORES)),
                               trace=trace)
    LAST_RESULTS = res
    yd = np.concatenate([res.results[k]["y"][None] for k in range(NCORES)],
                        axis=0).reshape(B, C, N)
    g = float(np.asarray(gamma, np.float32).reshape(-1)[0])
    y = g * yd.astype(np.float32) + x
    return y.reshape(B, C, H, W)


# revision 7
# speedup vs baseline: 1.7391x; 1.2541x over previous
"""Trainium2 Bass kernel for the channel-attention module.

Reference computation (per batch item, C=256 channels, N=4096 pixels):
    q = wq@x + bq; k = wk@x + bk; v = wv@x + bv          (1x1 convs)
    energy = q @ k^T                 [C, C]
    attn = softmax(energy, -1)
    out = attn @ v                   [C, N]
    y = gamma*out + x

Algorithm (algebraically identical, far less PE work):
    G' = [[x x^T, s], [s^T, N]]  (s = row sums of x)  -- Gram matrix, 257x257
    energy = wq' G' wk'^T   where wq' = [wq | bq], wk' = [wk | bk]
    attn = softmax(energy)
    out_dev = 16*(attn wv) x + 16*(attn bv) 1^T    (returned fp16)
    y = (gamma/16)*out_dev + x                      (host, fp32)

Key layout/precision choices:
  * Host supplies x^T pre-transposed with a ones column appended, so the
    Gram matmul also produces the row sums s (no DVE reduce, no DMA
    transpose on device).
  * All input DMAs go on ONE queue in need-order (xt chunks, then
    weights, then x8): a single queue runs at full per-core HBM rate,
    so the first gram chunk lands ~1.5us in and gram paces behind the
    feed instead of waiting for everything.
  * Gram is symmetric: row-block 1 is computed only for cols 128:258 and
    the (1,0) block is reconstructed with one PE transpose.
  * The final (attn wv) @ x matmul runs in fp8-e4m3 with DoubleRow perf
    mode (256-deep contraction per instruction, 2x PE rate). (attn wv)
    is scaled by 16 before the fp8 cast so its entries sit in e4m3's
    normal range; the host divides by 16 (exact). x is sent as a
    separate fp8 copy. The x passthrough and gamma scaling happen on
    the host in fp32, so fp8 never touches the dominant x term.
  * energy path stays fp16.

Sharding: data-parallel over batch B=16 across 8 cores (2 items/core).
"""

import os
import sys

sys.path.insert(0, "/opt/trn_rl_repo")

from contextlib import ExitStack

import ml_dtypes
import numpy as np

import concourse.bacc as bacc
import concourse.tile as tile
from concourse import masks, mybir
from concourse.bass_utils import run_bass_kernel_spmd

F32 = mybir.dt.float32
F16 = mybir.dt.float16
F8 = mybir.dt.float8e4

B, C, H, W = 16, 256, 64, 64
N = H * W                 # 4096
NCORES = 8
PB = B // NCORES          # batch items per core
P = 128                   # partitions
CT = C // P               # 2 channel tiles
NT = N // P               # 32 pixel tiles
CC = 258                  # per-pixel-tile row width: 256 ch + [1, 0]
NCH = 4                   # xt DMA chunks per item
NTC = NT // NCH           # pixel tiles per chunk (8)
FD = 512                  # free-dim per DoubleRow matmul (one PSUM bank)
OD = 1024                 # psum out tile width (2 banks, 2 matmuls)
ASC = 16.0                # fp8 prescale for (attn wv); host divides out

# wpack column layout (fp16, packed on host into [128, WCOLS]):
_WQ0, _WQ1 = 0, 256              # wq^T rows 0:128 / 128:256   [128,256] each
_WK0, _WK1 = 512, 768            # wk^T rows 0:128 / 128:256
_WV0, _WV1 = 1024, 1282         # [wv | 16*bv | 0] rows 0:128/128:256 [128,258]
_BQ = 1540                       # rows 0:2: [bq; 0]            [2,256]
_BK = 1796                       # rows 0:2: [bk; 0]            [2,256]
_NC = 2052                       # rows 0:2: [float(N); 0]      [2,1]
WCOLS = 2056


def _emit_core_program(nc, tc, ctx, xt_in, x8_in, wpack, y_out):
    sb1 = ctx.enter_context(tc.tile_pool(name="sb1", bufs=1))
    xtp = ctx.enter_context(tc.tile_pool(name="xtp", bufs=NCH * PB))
    x8p = ctx.enter_context(tc.tile_pool(name="x8p", bufs=PB))
    gsb = ctx.enter_context(tc.tile_pool(name="gsb", bufs=2 * PB))
    smp = ctx.enter_context(tc.tile_pool(name="smp", bufs=10))
    ysp = ctx.enter_context(tc.tile_pool(name="ysp", bufs=6))
    # PSUM pools: psm 2 + psb 2 + pso 4 = 8 banks
    psm = ctx.enter_context(tc.tile_pool(name="psm", bufs=2, space="PSUM"))
    psb = ctx.enter_context(tc.tile_pool(name="psb", bufs=2, space="PSUM"))
    pso = ctx.enter_context(tc.tile_pool(name="pso", bufs=2, space="PSUM"))

    # ---- all input DMAs on the sync queue, in need-order ----
    xt = []
    for b in range(PB):
        xt.append([xtp.tile([P, NTC * CC], F16, tag="xt", name=f"xt{b}_{ch}")
                   for ch in range(NCH)])
    for b in range(PB):
        for ch in range(NCH):
            nc.sync.dma_start(
                out=xt[b][ch],
                in_=xt_in[b, :, ch * NTC * CC:(ch + 1) * NTC * CC])
    wt = sb1.tile([P, WCOLS], F16)
    nc.sync.dma_start(out=wt, in_=wpack[:, :])
    x8s = []
    for b in range(PB):
        x8 = x8p.tile([P, CT, N], F8, tag="x8", name=f"x8_{b}")
        for ct in range(CT):
            nc.sync.dma_start(out=x8[:, ct, :],
                              in_=x8_in[b, ct * P:(ct + 1) * P, :])
        x8s.append(x8)

    # ---- constants ----
    ident_f = sb1.tile([P, P], F32)
    masks.make_identity(nc, ident_f[:, :])
    ident = sb1.tile([P, P], F16)
    nc.vector.tensor_copy(ident, ident_f)

    wq_k = [wt[:, _WQ0:_WQ0 + 256], wt[:, _WQ1:_WQ1 + 256],
            wt[0:2, _BQ:_BQ + 256]]
    wk_k = [wt[:, _WK0:_WK0 + 256], wt[:, _WK1:_WK1 + 256],
            wt[0:2, _BK:_BK + 256]]
    wv_t = [wt[:, _WV0:_WV0 + 258], wt[:, _WV1:_WV1 + 258]]

    st = [dict() for _ in range(PB)]

    # ---- phase A: gram matmuls (PE), chunk-paced ----
    for b in range(PB):
        s = st[b]
        with nc.named_scope("gram"):
            # both accumulators packed into one PSUM bank (258 + 130 f32)
            gpsA = psm.tile([P, 402], F32, tag="sm", name=f"gps{b}")
            gps0 = gpsA[:, 0:CC]
            gps1 = gpsA[:, 272:272 + CC - P]
            for nt in range(NT):
                ch, off = nt // NTC, nt % NTC
                xc = xt[b][ch]
                nc.tensor.matmul(gps0, xc[:, off * CC:off * CC + P],
                                 xc[:, off * CC:(off + 1) * CC],
                                 start=(nt == 0), stop=(nt == NT - 1))
                nc.tensor.matmul(gps1, xc[:, off * CC + P:off * CC + 2 * P],
                                 xc[:, off * CC + P:(off + 1) * CC],
                                 start=(nt == 0), stop=(nt == NT - 1))
            s["gps"] = (gps0, gps1)

    # ---- phase B: G assembly (direct copies first, then transposes) ----
    for b in range(PB):
        s = st[b]
        gps0, gps1 = s["gps"]
        with nc.named_scope("gass"):
            g0 = gsb.tile([P, CC], F16, tag="g", name=f"g{b}_0")
            nc.vector.tensor_copy(g0, gps0)
            g1 = gsb.tile([P, CC], F16, tag="g", name=f"g{b}_1")
            nc.vector.tensor_copy(g1[:, P:CC], gps1)
            s["g"] = (g0, g1)
    for b in range(PB):
        s = st[b]
        g0, g1 = s["g"]
        with nc.named_scope("gass"):
            # block (1,0) = block (0,1)^T via PE transpose
            tp10 = psm.tile([P, P], F16, tag="sm", name=f"tp10_{b}")
            nc.tensor.transpose(tp10, g0[:, P:2 * P], ident)
            nc.vector.tensor_copy(g1[:, 0:P], tp10)
            # g2 [2, 257] = [[s^T, N], [0, 0]] via PE transpose of s cols
            g2 = gsb.tile([2, 257], F16, tag="g2", name=f"g2_{b}")
            for ct in range(CT):
                g = (g0, g1)[ct]
                sp = psm.tile([2, P], F16, tag="sm", name=f"sp{b}_{ct}")
                nc.tensor.transpose(sp, g[:, 256:258], ident)
                nc.vector.tensor_copy(g2[0:2, ct * P:(ct + 1) * P], sp)
            nc.vector.tensor_copy(g2[0:2, 256:257], wt[0:2, _NC:_NC + 1])
            s["gk"] = (g0, g1, g2)

    # ---- phase C: T = (wq' G')^T and E = energy ----
    for b in range(PB):
        s = st[b]
        gk = s["gk"]
        with nc.named_scope("energy"):
            ttp = psb.tile([P, 2 * 256], F32, tag="big", name=f"ttp{b}")
            for mt in range(CT):
                for kt in range(3):
                    lhs = gk[kt][:, mt * P:(mt + 1) * P] if kt < 2 \
                        else gk[2][0:2, mt * P:(mt + 1) * P]
                    nc.tensor.matmul(ttp[:, mt * 256:(mt + 1) * 256],
                                     lhs, wq_k[kt],
                                     start=(kt == 0), stop=(kt == 2))
            tt2p = psm.tile([1, 256], F32, tag="sm", name=f"tt2p{b}")
            for kt in range(3):
                lhs = gk[kt][:, 256:257] if kt < 2 else gk[2][0:2, 256:257]
                nc.tensor.matmul(tt2p, lhs, wq_k[kt],
                                 start=(kt == 0), stop=(kt == 2))
            tt = []
            for mt in range(CT):
                t = gsb.tile([P, 256], F16, tag="tt", name=f"tt{b}_{mt}")
                nc.vector.tensor_copy(t, ttp[:, mt * 256:(mt + 1) * 256])
                tt.append(t)
            t2 = gsb.tile([1, 256], F16, tag="tt2", name=f"tt2_{b}")
            nc.vector.tensor_copy(t2, tt2p)
            tt.append(t2)

            ep = psb.tile([P, 2 * 256], F32, tag="big", name=f"ep{b}")
            for it in range(CT):
                for kt in range(3):
                    lhs = tt[kt][:, it * P:(it + 1) * P] if kt < 2 \
                        else tt[2][0:1, it * P:(it + 1) * P]
                    nc.tensor.matmul(ep[:, it * 256:(it + 1) * 256],
                                     lhs, wk_k[kt][0:1, :] if kt == 2
                                     else wk_k[kt],
                                     start=(kt == 0), stop=(kt == 2))
            s["ep"] = ep

    # ---- phase D: softmax, attn^T, (attn wv) in fp8, abv ----
    for b in range(PB):
        s = st[b]
        ep = s["ep"]
        with nc.named_scope("softmax"):
            attn = []
            for it in range(CT):
                eslice = ep[:, it * 256:(it + 1) * 256]
                nmx = smp.tile([P, 1], F32, tag="nmx", name=f"nmx{b}_{it}")
                nc.vector.tensor_reduce(
                    nmx, eslice, axis=mybir.AxisListType.X,
                    op=mybir.AluOpType.max, negate=True)
                at = smp.tile([P, 256], F16, tag="attn", name=f"at{b}_{it}")
                rs = smp.tile([P, 1], F32, tag="rs", name=f"rs{b}_{it}")
                nc.scalar.activation(
                    out=at, in_=eslice,
                    func=mybir.ActivationFunctionType.Exp,
                    bias=nmx, scale=1.0, accum_out=rs)
                ri = smp.tile([P, 1], F32, tag="ri", name=f"ri{b}_{it}")
                nc.vector.reciprocal(ri, rs)
                nc.vector.tensor_scalar_mul(at, at, ri)
                attn.append(at)
            attnT = [smp.tile([P, 256], F16, tag="attnT", name=f"aT{b}_{jt}")
                     for jt in range(CT)]
            for it in range(CT):
                for jt in range(CT):
                    tp = psm.tile([P, P], F16, tag="sm", name=f"tA{b}{jt}{it}")
                    nc.tensor.transpose(
                        tp, attn[it][:, jt * P:(jt + 1) * P], ident)
                    nc.vector.tensor_copy(
                        attnT[jt][:, it * P:(it + 1) * P], tp)

        with nc.named_scope("attn_wv"):
            # at8[p, ct, o] = 16*(attn wv)[o, ct*128+p] in fp8
            at8 = smp.tile([P, CT, 256], F8, tag="at8", name=f"at8_{b}")
            ap_ = psb.tile([P, 2 * 256], F32, tag="big", name=f"ap{b}")
            for mt in range(CT):
                for jt in range(CT):
                    nc.tensor.matmul(
                        ap_[:, mt * 256:(mt + 1) * 256],
                        wv_t[jt][:, mt * P:(mt + 1) * P], attnT[jt],
                        start=(jt == 0), stop=(jt == 1))
            for mt in range(CT):
                nc.vector.tensor_scalar_mul(at8[:, mt, :],
                                            ap_[:, mt * 256:(mt + 1) * 256],
                                            ASC)
            # abv[it] = 16*(attn bv) column [128,1] (bv pre-scaled in wpack)
            abv = []
            for it in range(CT):
                avp = psm.tile([P, 1], F32, tag="sm", name=f"avp{b}_{it}")
                for jt in range(CT):
                    nc.tensor.matmul(avp,
                                     attnT[jt][:, it * P:(it + 1) * P],
                                     wv_t[jt][:, 256:257],
                                     start=(jt == 0), stop=(jt == 1))
                ac = smp.tile([P, 1], F32, tag="abv", name=f"abv{b}_{it}")
                nc.vector.tensor_copy(ac, avp)
                abv.append(ac)
            s["at8"], s["abv"] = at8, abv

    # ---- phase E: out = 16*(attn wv) x + abv (fp8 DoubleRow) ----
    for b in range(PB):
        s = st[b]
        at8, abv, x8 = s["at8"], s["abv"], x8s[b]
        with nc.named_scope("out_mm"):
            for it in range(CT):
                lhsT = at8[:, :, it * P:(it + 1) * P]
                for og in range(N // OD):
                    op = pso.tile([P, OD], F32, tag="out",
                                  name=f"op{b}_{it}_{og}")
                    for h in range(OD // FD):
                        nch = og * (OD // FD) + h
                        nc.tensor.matmul(
                            op[:, h * FD:(h + 1) * FD], lhsT,
                            x8[:, :, nch * FD:(nch + 1) * FD],
                            start=True, stop=True,
                            perf_mode=mybir.MatmulPerfMode.DoubleRow)
                    ysb = ysp.tile([P, OD], F16, tag="ysb",
                                   name=f"ysb{b}_{it}_{og}")
                    if og % 2 == 0:
                        nc.scalar.add(ysb, op, add=abv[it])
                    else:
                        nc.vector.tensor_scalar_add(ysb, op, abv[it])
                    nc.sync.dma_start(
                        out=y_out[b, it * P:(it + 1) * P,
                                  og * OD:(og + 1) * OD],
                        in_=ysb)


_CACHE = {}
LAST_RESULTS = None


def _build():
    if "nc" in _CACHE:
        return _CACHE["nc"]
    nc = bacc.Bacc()
    xt_in = nc.declare_dram_parameter("xt", [PB, P, NT * CC], F16,
                                      isOutput=False)
    x8_in = nc.declare_dram_parameter("x8", [PB, C, N], F8, isOutput=False)
    wpack = nc.declare_dram_parameter("wpack", [P, WCOLS], F16,
                                      isOutput=False)
    y_out = nc.declare_dram_parameter("y", [PB, C, N], F16, isOutput=True)
    with ExitStack() as ctx:
        tc = ctx.enter_context(tile.TileContext(nc))
        _emit_core_program(nc, tc, ctx, xt_in, x8_in, wpack, y_out)
    nc.compile()
    _CACHE["nc"] = nc
    return nc


def _pack_weights(wq, bq, wk, bk, wv, bv):
    wp = np.zeros((P, WCOLS), np.float16)
    wqT = np.ascontiguousarray(wq.T).astype(np.float16)
    wkT = np.ascontiguousarray(wk.T).astype(np.float16)
    wp[:, _WQ0:_WQ0 + 256] = wqT[0:P]
    wp[:, _WQ1:_WQ1 + 256] = wqT[P:C]
    wp[:, _WK0:_WK0 + 256] = wkT[0:P]
    wp[:, _WK1:_WK1 + 256] = wkT[P:C]
    wvp = np.concatenate([wv, ASC * bv[:, None]],
                         axis=1).astype(np.float16)  # [256, 257]
    wp[:, _WV0:_WV0 + 257] = wvp[0:P]
    wp[:, _WV1:_WV1 + 257] = wvp[P:C]
    wp[0, _BQ:_BQ + 256] = bq.astype(np.float16)
    wp[0, _BK:_BK + 256] = bk.astype(np.float16)
    wp[0, _NC] = np.float16(float(N))
    return wp


def kernel(x, wq, bq, wk, bk, wv, bv, gamma):
    global LAST_RESULTS
    x = np.ascontiguousarray(np.asarray(x, np.float32).reshape(B, C, N))
    x16 = x.astype(np.float16)
    # xt[b, p, nt, c] = x[b, c, nt*128+p]; col 256 = 1.0, col 257 = 0.0
    xt = np.zeros((B, P, NT, CC), np.float16)
    xt[:, :, :, :256] = x16.reshape(B, C, NT, P).transpose(0, 3, 2, 1)
    xt[:, :, :, 256] = np.float16(1.0)
    xt = np.ascontiguousarray(xt.reshape(B, P, NT * CC))
    x8 = x.astype(ml_dtypes.float8_e4m3)
    wp = _pack_weights(np.asarray(wq, np.float32), np.asarray(bq, np.float32),
                       np.asarray(wk, np.float32), np.asarray(bk, np.float32),
                       np.asarray(wv, np.float32), np.asarray(bv, np.float32))
    nc = _build()
    in_maps = []
    for k in range(NCORES):
        in_maps.append({
            "xt": np.ascontiguousarray(xt[k * PB:(k + 1) * PB]),
            "x8": np.ascontiguousarray(x8[k * PB:(k + 1) * PB]),
            "wpack": wp,
        })
    trace = bool(int(os.environ.get("KERNEL_TRACE", "0")))
    res = run_bass_kernel_spmd(nc, in_maps, core_ids=list(range(NCORES)),
                               trace=trace)
    LAST_RESULTS = res
    yd = np.concatenate([res.results[k]["y"][None] for k in range(NCORES)],
                        axis=0).reshape(B, C, N)
    g = float(np.asarray(gamma, np.float32).reshape(-1)[0])
    y = (g / ASC) * yd.astype(np.float32) + x
    return y.reshape(B, C, H, W)


# revision 8
# speedup vs baseline: 1.7744x; 1.0203x over previous
"""Trainium2 Bass kernel for the channel-attention module.

Reference computation (per batch item, C=256 channels, N=4096 pixels):
    q = wq@x + bq; k = wk@x + bk; v = wv@x + bv          (1x1 convs)
    energy = q @ k^T                 [C, C]
    attn = softmax(energy, -1)
    out = attn @ v                   [C, N]
    y = gamma*out + x

Algorithm (algebraically identical, far less PE work):
    G' = [[x x^T, s], [s^T, N]]  (s = row sums of x)  -- Gram matrix, 257x257
    energy = wq' G' wk'^T   where wq' = [wq | bq], wk' = [wk | bk]
    attn = softmax(energy)
    out_dev = 16*(attn wv) x + 16*(attn bv) 1^T    (returned fp16)
    y = (gamma/16)*out_dev + x                      (host, fp32)

Key layout/precision choices:
  * Host supplies x^T pre-transposed with a ones column appended, so the
    Gram matmul also produces the row sums s (no DVE reduce, no DMA
    transpose on device).
  * All input DMAs go on ONE queue in need-order (xt chunks, then
    weights, then x8): a single queue runs at full per-core HBM rate,
    so the first gram chunk lands ~1.5us in and gram paces behind the
    feed instead of waiting for everything.
  * Gram is symmetric: row-block 1 is computed only for cols 128:258 and
    the (1,0) block is reconstructed with one PE transpose.
  * The final (attn wv) @ x matmul runs in fp8-e4m3 with DoubleRow perf
    mode (256-deep contraction per instruction, 2x PE rate). (attn wv)
    is scaled by 16 before the fp8 cast so its entries sit in e4m3's
    normal range; the host divides by 16 (exact). x is sent as a
    separate fp8 copy. The x passthrough and gamma scaling happen on
    the host in fp32, so fp8 never touches the dominant x term.
  * energy path stays fp16.

Sharding: data-parallel over batch B=16 across 8 cores (2 items/core).
"""

import os
import sys

sys.path.insert(0, "/opt/trn_rl_repo")

from contextlib import ExitStack

import ml_dtypes
import numpy as np

import concourse.bacc as bacc
import concourse.tile as tile
from concourse import masks, mybir
from concourse.bass_utils import run_bass_kernel_spmd

F32 = mybir.dt.float32
F16 = mybir.dt.float16
F8 = mybir.dt.float8e4

B, C, H, W = 16, 256, 64, 64
N = H * W                 # 4096
NCORES = 8
PB = B // NCORES          # batch items per core
P = 128                   # partitions
CT = C // P               # 2 channel tiles
NT = N // P               # 32 pixel tiles
CC = 258                  # per-pixel-tile row width: 256 ch + [1, 0]
NCH = 4                   # xt DMA chunks per item
NTC = NT // NCH           # pixel tiles per chunk (8)
FD = 512                  # free-dim per DoubleRow matmul (one PSUM bank)
OD = 1024                 # psum out tile width (2 banks, 2 matmuls)
ASC = 16.0                # fp8 prescale for (attn wv); host divides out

# wpack column layout (fp16, packed on host into [128, WCOLS]):
_WQ0, _WQ1 = 0, 256              # wq^T rows 0:128 / 128:256   [128,256] each
_WK0, _WK1 = 512, 768            # wk^T rows 0:128 / 128:256
_WV0, _WV1 = 1024, 1282         # [wv | 16*bv | 0] rows 0:128/128:256 [128,258]
_BQ = 1540                       # rows 0:2: [bq; 0]            [2,256]
_BK = 1796                       # rows 0:2: [bk; 0]            [2,256]
_NC = 2052                       # rows 0:2: [float(N); 0]      [2,1]
WCOLS = 2056


def _emit_core_program(nc, tc, ctx, xt_in, x8_in, wpack, y_out):
    sb1 = ctx.enter_context(tc.tile_pool(name="sb1", bufs=1))
    xtp = ctx.enter_context(tc.tile_pool(name="xtp", bufs=NCH * PB))
    x8p = ctx.enter_context(tc.tile_pool(name="x8p", bufs=PB))
    gsb = ctx.enter_context(tc.tile_pool(name="gsb", bufs=2 * PB))
    smp = ctx.enter_context(tc.tile_pool(name="smp", bufs=10))
    ysp = ctx.enter_context(tc.tile_pool(name="ysp", bufs=6))
    # PSUM pools: psm 2 + psb 2 + pso 4 = 8 banks
    psm = ctx.enter_context(tc.tile_pool(name="psm", bufs=2, space="PSUM"))
    psb = ctx.enter_context(tc.tile_pool(name="psb", bufs=2, space="PSUM"))
    pso = ctx.enter_context(tc.tile_pool(name="pso", bufs=2, space="PSUM"))

    # ---- all input DMAs on the sync queue, in need-order ----
    xt = []
    for b in range(PB):
        xt.append([xtp.tile([P, NTC * CC], F16, tag="xt", name=f"xt{b}_{ch}")
                   for ch in range(NCH)])
    for b in range(PB):
        for ch in range(NCH):
            nc.sync.dma_start(
                out=xt[b][ch],
                in_=xt_in[b, :, ch * NTC * CC:(ch + 1) * NTC * CC])
    wt = sb1.tile([P, WCOLS], F16)
    nc.sync.dma_start(out=wt, in_=wpack[:, :])
    x8s = []
    for b in range(PB):
        x8 = x8p.tile([P, CT, N], F8, tag="x8", name=f"x8_{b}")
        for ct in range(CT):
            nc.sync.dma_start(out=x8[:, ct, :],
                              in_=x8_in[b, ct * P:(ct + 1) * P, :])
        x8s.append(x8)

    # ---- constants ----
    ident_f = sb1.tile([P, P], F32)
    masks.make_identity(nc, ident_f[:, :])
    ident = sb1.tile([P, P], F16)
    nc.vector.tensor_copy(ident, ident_f)

    wq_k = [wt[:, _WQ0:_WQ0 + 256], wt[:, _WQ1:_WQ1 + 256],
            wt[0:2, _BQ:_BQ + 256]]
    wk_k = [wt[:, _WK0:_WK0 + 256], wt[:, _WK1:_WK1 + 256],
            wt[0:2, _BK:_BK + 256]]
    wv_t = [wt[:, _WV0:_WV0 + 258], wt[:, _WV1:_WV1 + 258]]

    st = [dict() for _ in range(PB)]

    # ---- phase A: gram matmuls (PE), chunk-paced ----
    for b in range(PB):
        s = st[b]
        with nc.named_scope("gram"):
            # separate PSUM banks per accumulation group
            gps0 = psm.tile([P, CC], F32, tag="sm", name=f"gps{b}_0")
            gps1 = psm.tile([P, CC - P], F32, tag="sm", name=f"gps{b}_1")
            for nt in range(NT):
                ch, off = nt // NTC, nt % NTC
                xc = xt[b][ch]
                nc.tensor.matmul(gps0, xc[:, off * CC:off * CC + P],
                                 xc[:, off * CC:(off + 1) * CC],
                                 start=(nt == 0), stop=(nt == NT - 1))
                nc.tensor.matmul(gps1, xc[:, off * CC + P:off * CC + 2 * P],
                                 xc[:, off * CC + P:(off + 1) * CC],
                                 start=(nt == 0), stop=(nt == NT - 1))
            s["gps"] = (gps0, gps1)

    # ---- phase B: G assembly (direct copies first, then transposes) ----
    for b in range(PB):
        s = st[b]
        gps0, gps1 = s["gps"]
        with nc.named_scope("gass"):
            g0 = gsb.tile([P, CC], F16, tag="g", name=f"g{b}_0")
            nc.vector.tensor_copy(g0, gps0)
            g1 = gsb.tile([P, CC], F16, tag="g", name=f"g{b}_1")
            nc.vector.tensor_copy(g1[:, P:CC], gps1)
            s["g"] = (g0, g1)
    for b in range(PB):
        s = st[b]
        g0, g1 = s["g"]
        with nc.named_scope("gass"):
            # block (1,0) = block (0,1)^T via PE transpose
            tp10 = psm.tile([P, P], F16, tag="sm", name=f"tp10_{b}")
            nc.tensor.transpose(tp10, g0[:, P:2 * P], ident)
            nc.vector.tensor_copy(g1[:, 0:P], tp10)
            # g2 [2, 257] = [[s^T, N], [0, 0]] via PE transpose of s cols
            g2 = gsb.tile([2, 257], F16, tag="g2", name=f"g2_{b}")
            for ct in range(CT):
                g = (g0, g1)[ct]
                sp = psm.tile([2, P], F16, tag="sm", name=f"sp{b}_{ct}")
                nc.tensor.transpose(sp, g[:, 256:258], ident)
                nc.vector.tensor_copy(g2[0:2, ct * P:(ct + 1) * P], sp)
            nc.vector.tensor_copy(g2[0:2, 256:257], wt[0:2, _NC:_NC + 1])
            s["gk"] = (g0, g1, g2)

    # ---- phase C: T = (wq' G')^T and E = energy ----
    for b in range(PB):
        s = st[b]
        gk = s["gk"]
        with nc.named_scope("energy"):
            ttp = psb.tile([P, 2 * 256], F32, tag="big", name=f"ttp{b}")
            for mt in range(CT):
                for kt in range(3):
                    lhs = gk[kt][:, mt * P:(mt + 1) * P] if kt < 2 \
                        else gk[2][0:2, mt * P:(mt + 1) * P]
                    nc.tensor.matmul(ttp[:, mt * 256:(mt + 1) * 256],
                                     lhs, wq_k[kt],
                                     start=(kt == 0), stop=(kt == 2))
            tt2p = psm.tile([1, 256], F32, tag="sm", name=f"tt2p{b}")
            for kt in range(3):
                lhs = gk[kt][:, 256:257] if kt < 2 else gk[2][0:2, 256:257]
                nc.tensor.matmul(tt2p, lhs, wq_k[kt],
                                 start=(kt == 0), stop=(kt == 2))
            tt = []
            for mt in range(CT):
                t = gsb.tile([P, 256], F16, tag="tt", name=f"tt{b}_{mt}")
                nc.vector.tensor_copy(t, ttp[:, mt * 256:(mt + 1) * 256])
                tt.append(t)
            t2 = gsb.tile([1, 256], F16, tag="tt2", name=f"tt2_{b}")
            nc.vector.tensor_copy(t2, tt2p)
            tt.append(t2)

            ep = psb.tile([P, 2 * 256], F32, tag="big", name=f"ep{b}")
            for it in range(CT):
                for kt in range(3):
                    lhs = tt[kt][:, it * P:(it + 1) * P] if kt < 2 \
                        else tt[2][0:1, it * P:(it + 1) * P]
                    nc.tensor.matmul(ep[:, it * 256:(it + 1) * 256],
                                     lhs, wk_k[kt][0:1, :] if kt == 2
                                     else wk_k[kt],
                                     start=(kt == 0), stop=(kt == 2))
            s["ep"] = ep

    # ---- phase D: softmax, attn^T, (attn wv) in fp8, abv ----
    for b in range(PB):
        s = st[b]
        ep = s["ep"]
        with nc.named_scope("softmax"):
            attn = []
            for it in range(CT):
                eslice = ep[:, it * 256:(it + 1) * 256]
                nmx = smp.tile([P, 1], F32, tag="nmx", name=f"nmx{b}_{it}")
                nc.vector.tensor_reduce(
                    nmx, eslice, axis=mybir.AxisListType.X,
                    op=mybir.AluOpType.max, negate=True)
                at = smp.tile([P, 256], F16, tag="attn", name=f"at{b}_{it}")
                rs = smp.tile([P, 1], F32, tag="rs", name=f"rs{b}_{it}")
                nc.scalar.activation(
                    out=at, in_=eslice,
                    func=mybir.ActivationFunctionType.Exp,
                    bias=nmx, scale=1.0, accum_out=rs)
                ri = smp.tile([P, 1], F32, tag="ri", name=f"ri{b}_{it}")
                nc.vector.reciprocal(ri, rs)
                nc.vector.tensor_scalar_mul(at, at, ri)
                attn.append(at)
            attnT = [smp.tile([P, 256], F16, tag="attnT", name=f"aT{b}_{jt}")
                     for jt in range(CT)]
            for it in range(CT):
                for jt in range(CT):
                    tp = psm.tile([P, P], F16, tag="sm", name=f"tA{b}{jt}{it}")
                    nc.tensor.transpose(
                        tp, attn[it][:, jt * P:(jt + 1) * P], ident)
                    nc.vector.tensor_copy(
                        attnT[jt][:, it * P:(it + 1) * P], tp)

        with nc.named_scope("attn_wv"):
            # at8[p, ct, o] = 16*(attn wv)[o, ct*128+p] in fp8
            at8 = smp.tile([P, CT, 256], F8, tag="at8", name=f"at8_{b}")
            ap_ = psb.tile([P, 2 * 256], F32, tag="big", name=f"ap{b}")
            for mt in range(CT):
                for jt in range(CT):
                    nc.tensor.matmul(
                        ap_[:, mt * 256:(mt + 1) * 256],
                        wv_t[jt][:, mt * P:(mt + 1) * P], attnT[jt],
                        start=(jt == 0), stop=(jt == 1))
            for mt in range(CT):
                nc.vector.tensor_scalar_mul(at8[:, mt, :],
                                            ap_[:, mt * 256:(mt + 1) * 256],
                                            ASC)
            # abv[it] = 16*(attn bv) column [128,1] (bv pre-scaled in wpack)
            abv = []
            for it in range(CT):
                avp = psm.tile([P, 1], F32, tag="sm", name=f"avp{b}_{it}")
                for jt in range(CT):
                    nc.tensor.matmul(avp,
                                     attnT[jt][:, it * P:(it + 1) * P],
                                     wv_t[jt][:, 256:257],
                                     start=(jt == 0), stop=(jt == 1))
                ac = smp.tile([P, 1], F32, tag="abv", name=f"abv{b}_{it}")
                nc.vector.tensor_copy(ac, avp)
                abv.append(ac)
            s["at8"], s["abv"] = at8, abv

    # ---- phase E: out = 16*(attn wv) x + abv (fp8 DoubleRow) ----
    for b in range(PB):
        s = st[b]
        at8, abv, x8 = s["at8"], s["abv"], x8s[b]
        with nc.named_scope("out_mm"):
            for it in range(CT):
                lhsT = at8[:, :, it * P:(it + 1) * P]
                for og in range(N // OD):
                    op = pso.tile([P, OD], F32, tag="out",
                                  name=f"op{b}_{it}_{og}")
                    for h in range(OD // FD):
                        nch = og * (OD // FD) + h
                        nc.tensor.matmul(
                            op[:, h * FD:(h + 1) * FD], lhsT,
                            x8[:, :, nch * FD:(nch + 1) * FD],
                            start=True, stop=True,
                            perf_mode=mybir.MatmulPerfMode.DoubleRow)
                    ysb = ysp.tile([P, OD], F16, tag="ysb",
                                   name=f"ysb{b}_{it}_{og}")
                    if og % 2 == 0:
                        nc.scalar.add(ysb, op, add=abv[it])
                    else:
                        nc.vector.tensor_scalar_add(ysb, op, abv[it])
                    nc.sync.dma_start(
                        out=y_out[b, it * P:(it + 1) * P,
                                  og * OD:(og + 1) * OD],
                        in_=ysb)


_CACHE = {}
LAST_RESULTS = None


def _build():
    if "nc" in _CACHE:
        return _CACHE["nc"]
    nc = bacc.Bacc()
    xt_in = nc.declare_dram_parameter("xt", [PB, P, NT * CC], F16,
                                      isOutput=False)
    x8_in = nc.declare_dram_parameter("x8", [PB, C, N], F8, isOutput=False)
    wpack = nc.declare_dram_parameter("wpack", [P, WCOLS], F16,
                                      isOutput=False)
    y_out = nc.declare_dram_parameter("y", [PB, C, N], F16, isOutput=True)
    with ExitStack() as ctx:
        tc = ctx.enter_context(tile.TileContext(nc))
        _emit_core_program(nc, tc, ctx, xt_in, x8_in, wpack, y_out)
    nc.compile()
    _CACHE["nc"] = nc
    return nc


def _pack_weights(wq, bq, wk, bk, wv, bv):
    wp = np.zeros((P, WCOLS), np.float16)
    wqT = np.ascontiguousarray(wq.T).astype(np.float16)
    wkT = np.ascontiguousarray(wk.T).astype(np.float16)
    wp[:, _WQ0:_WQ0 + 256] = wqT[0:P]
    wp[:, _WQ1:_WQ1 + 256] = wqT[P:C]
    wp[:, _WK0:_WK0 + 256] = wkT[0:P]
    wp[:, _WK1:_WK1 + 256] = wkT[P:C]
    wvp = np.concatenate([wv, ASC * bv[:, None]],
                         axis=1).astype(np.float16)  # [256, 257]
    wp[:, _WV0:_WV0 + 257] = wvp[0:P]
    wp[:, _WV1:_WV1 + 257] = wvp[P:C]
    wp[0, _BQ:_BQ + 256] = bq.astype(np.float16)
    wp[0, _BK:_BK + 256] = bk.astype(np.float16)
    wp[0, _NC] = np.float16(float(N))
    return wp


def kernel(x, wq, bq, wk, bk, wv, bv, gamma):
    global LAST_RESULTS
    x = np.ascontiguousarray(np.asarray(x, np.float32).reshape(B, C, N))
    x16 = x.astype(np.float16)
    # xt[b, p, nt, c] = x[b, c, nt*128+p]; col 256 = 1.0, col 257 = 0.0
    xt = np.zeros((B, P, NT, CC), np.float16)
    xt[:, :, :, :256] = x16.reshape(B, C, NT, P).transpose(0, 3, 2, 1)
    xt[:, :, :, 256] = np.float16(1.0)
    xt = np.ascontiguousarray(xt.reshape(B, P, NT * CC))
    x8 = x.astype(ml_dtypes.float8_e4m3)
    wp = _pack_weights(np.asarray(wq, np.float32), np.asarray(bq, np.float32),
                       np.asarray(wk, np.float32), np.asarray(bk, np.float32),
                       np.asarray(wv, np.float32), np.asarray(bv, np.float32))
    nc = _build()
    in_maps = []
    for k in range(NCORES):
        in_maps.append({
            "xt": np.ascontiguousarray(xt[k * PB:(k + 1) * PB]),
            "x8": np.ascontiguousarray(x8[k * PB:(k + 1) * PB]),
            "wpack": wp,
        })
    trace = bool(int(os.environ.get("KERNEL_TRACE", "0")))
    res = run_bass_kernel_spmd(nc, in_maps, core_ids=list(range(NCORES)),
                               trace=trace)
    LAST_RESULTS = res
    yd = np.concatenate([res.results[k]["y"][None] for k in range(NCORES)],
                        axis=0).reshape(B, C, N)
    g = float(np.asarray(gamma, np.float32).reshape(-1)[0])
    y = (g / ASC) * yd.astype(np.float32) + x
    return y.reshape(B, C, H, W)
